# revision 93
# baseline (speedup 1.0000x reference)
"""Trainium2 Bass kernel for the dual-softmax interaction (BiDAF-style) layer.

fp8-e4m3 DoubleRow attention products + fp8 hi/lo split-K logits.

Key tricks vs the bf16 baseline:
- Logit matmul runs in fp8 DoubleRow with a 3-slot hi/lo residual
  decomposition (hi*hi + lo*hi + hi*lo, per-h sqrt|w_m| balanced), ~0.01
  worst-case logit error at 0.75x the fp16 PE cost.
- The gamma (=exp(sq)) column-scaling is folded INTO S by stealing the
  h-slot with the smallest |w_m|: that lhsT row becomes ones and the rhs
  row ln(gamma), so S''[i,c] = gamma_c*exp(tri) comes out of the exp
  directly.  This removes the gamma column from the attention rhs and
  merges P1|P2 into ONE 512-wide DoubleRow matmul.
- Both softmax normalizers come out of matmuls: the B-side as ownd's
  delta column (Q1 psum col 256), the A-side as comb's ones-column via a
  tiny N=1 matmul chasing each P matmul — psum rows and normalizer share
  the SwInterleave row order, so no reversal fixup anywhere.
- The exp of the 2048x2048 logit matrix is spread over THREE engines
  (Act exp / DVE 2-pass Schraudolph / DVE pass 1 + Pool pass 2), sized
  so every engine stays just below the PE's 20.5us of matmul work.
  GPSIMD cannot touch PSUM on real hw, so Pool only sees the SBUF i32
  intermediate; hw tensor_scalar has no divide, so scales go through
  DVE reciprocal + multiply.
- PE p-state warmup matmuls burn the cost model's 3us clock ramp while
  the first input DMAs are in flight; input DMAs stream in need-by
  order (HWDGE is a serial ~630ns/DMA device, transfers ~2.84us/MB).
- S'' is stored fp8 with columns in sigma order (c=2u -> q=u,
  c=2u+1 -> q=1024+u); the uint16 pair view of S'' is DMA-transposed so
  T lands as [p, st, i, r] with q = st*128 + p + 1024r.  The P matmuls
  consume T via MatmulPerfMode.DoubleRowSwInterleave (pairs adjacent,
  columns reversed = exactly the SwInterleave weight layout); rhs pairs
  are comb strip pairs (st,st+8).  S/T live in per-strip(-pair) tiles.

Math per core (own/oth swapped between the two cores of a batch):
    S''[i,c]  = gamma_c exp(sum_h (own*w_m)[i,h] oth[sigma(c),h])
    Q1[q,:]   = S''^T @ [own*delta | delta]  -> gamma_q*(B1 numer|norm)
                (gamma_q cancels in the ratio; q1g = 32*B1_normalized)
    P[i,:]    = T''^T @ [oth | q1g | 1]      -> A1|32*A2 numers | A-norm
Device returns [A1 | 32*A2] bf16; host assembles G = [E, A1, A2, E*A1,
E*A2] f32 from the original f32 inputs.
"""

import sys

if "/opt/trn_rl_repo" not in sys.path:
    sys.path.insert(0, "/opt/trn_rl_repo")

import numpy as np

import concourse.bass as bass  # noqa: F401  (registers AP machinery)
import concourse.mybir as mybir
import concourse.tile as tile
from concourse import bacc
from concourse.bass_utils import run_bass_kernel_spmd

B, SEQ, H = 4, 2048, 256
P = 128
NS = SEQ // P  # 16 i-strips
NSQ = NS // 2  # 8 q-pair strips
RC = 260  # ownd row: 256 delta*own + 1 delta + 3 pad
RW = 520  # comb row: 256 oth + 256 q1g + 1 ones (A-norm) + 7 pad
QA = 260  # qacc row: 257 used + 3 pad
QS = 32.0  # q1g pre-scale keeping B1 values in fp8 normal range
LOG2E = float(np.log2(np.e))
LN2 = float(np.log(2.0))
SB = 0.0435  # Schraudolph bias (truncating f32->i32 convert)
YOFF = 127.0 - SB  # folded into the ln(gamma) row; psum = log2-logits + YOFF
N_CORES = 8

f32 = mybir.dt.float32
bf16 = mybir.dt.bfloat16
fp8 = mybir.dt.float8e4
u16 = mybir.dt.uint16
i32 = mybir.dt.int32
fp8_np = mybir.dt.np(fp8)
bf16_np = mybir.dt.np(bf16)
DR = mybir.MatmulPerfMode.DoubleRow
DRS = mybir.MatmulPerfMode.DoubleRowSwInterleave

def emit_kernel(nc, tc, u_lhs, u_rhs, ownd, comb0, g):
    Exp = mybir.ActivationFunctionType.Exp
    Copy = mybir.ActivationFunctionType.Copy
    add = mybir.AluOpType.add

    with (
        tc.tile_pool(name="big", bufs=1) as big,
        tc.tile_pool(name="small", bufs=1) as small,
    ):
        # S and T are split into per-strip(-pair) tiles so downstream reads
        # depend only on the strips they touch, not on the whole U phase
        # (a single big tile made Q1's first matmul wait for the LAST exp).
        Spair = [
            big.tile([P, 2, SEQ], fp8, name=f"Spair{k}") for k in range(NS // 2)
        ]
        Tt = [big.tile([P, NSQ, P, 2], fp8, name=f"Tt{s}") for s in range(NS)]
        comb = big.tile([P, NS, RW], fp8)  # strip k: rows q=k*128+p
        ownd_sb = big.tile([P, NS, RC], fp8)
        # U operands in fp8 hi+lo split-K: k-tiles [k0hi, k1hi, k0lo, k1lo]
        lhsU = big.tile([P, 4, SEQ], fp8)
        rhsU = big.tile([P, 4, SEQ], fp8)
        ebias = small.tile([P, 1], f32)  # -YOFF*ln2 activation bias
        nc.vector.memset(ebias[:], -YOFF * LN2)

        # PE p-state warmup: the cost model runs matmuls at half speed until
        # the PE has been continuously busy for 3us.  Burn that ramp on dummy
        # bf16 matmuls over a memset tile while the first input DMAs are in
        # flight, so every real matmul runs at full clock.  The memset runs
        # on the Pool engine, whose queue is free right after the preamble.
        wz = small.tile([P, 512], bf16)
        nc.gpsimd.memset(wz[:], 0.0)
        with tc.tile_pool(name="warm", bufs=1, space="PSUM") as warm:
            wps = warm.tile([P, 512], f32, tag="warm")
            for _ in range(5):
                nc.tensor.matmul(
                    wps[:], lhsT=wz[:, 0:P], rhs=wz[:], start=True, stop=True
                )
            # consume so the pool close doesn't warn about unread psum
            nc.vector.tensor_scalar_mul(wz[:, 0:1], wps[:, 0:1], 0.0)

        # Everything streams through SP/HWDGE (serial, ~630ns/DMA descriptor
        # gen, wire ~2.84us/MB) in the order the chunk-major U phase below
        # consumes it: rhs hi c0, lhs c0, rhs lo c0, then alternating
        # rhs/lhs columns, and last the Q1/P operands.
        uview = lambda ap: ap.rearrange("p (a b) -> p a b", a=4)
        nc.sync.dma_start(rhsU[:, 0:2, 0:512], uview(u_rhs)[:, 0:2, 0:512])
        nc.sync.dma_start(lhsU[:, :, 0:512], uview(u_lhs)[:, :, 0:512])
        nc.sync.dma_start(rhsU[:, 2:4, 0:512], uview(u_rhs)[:, 2:4, 0:512])
        for c4 in range(1, 4):
            csl = slice(c4 * 512, (c4 + 1) * 512)
            nc.sync.dma_start(rhsU[:, :, csl], uview(u_rhs)[:, :, csl])
            nc.sync.dma_start(lhsU[:, :, csl], uview(u_lhs)[:, :, csl])
        nc.sync.dma_start(ownd_sb.rearrange("p s c -> p (s c)"), ownd[:])
        # full-width contiguous load (strided-dst DMA scrambles on real hw)
        nc.sync.dma_start(comb.rearrange("p s c -> p (s c)"), comb0[:])



        # ---- U phase: S'' = gamma*exp(tri) fp8; T = pair-T ----
        # psum holds y' = log2(S'') + YOFF.  Exp spreads over three engines:
        # 'A' tiles go to the Act engine (Exp with scale=ln2, bias=-YOFF*ln2),
        # 'D' tiles to the DVE (2-pass Schraudolph: truncating f32->i32 of
        # y'*2^23, then a bitcast-f32 copy to fp8), and 'Y' tiles run pass 1
        # on the DVE but hand pass 2 (SBUF->SBUF) to the Pool engine.  GPSIMD
        # cannot touch PSUM on real hw, so Pool can only ever see the i32
        # intermediate.  No row-sum accumulation here: the A-softmax
        # normalizer comes out of the P matmul's ones-column (comb col 512).
        import os as _os
        _odd = list("YYDYYYAYYDYYYAYY")  # odd slots: Y12 D2 A2 (swept best)
        _dflt = "".join(c for pair in zip("A" * 16, _odd) for c in pair)
        ASSIGN = list(_os.environ.get("KASSIGN", _dflt))[:2 * NS]
        with (
            tc.tile_pool(name="upsum", bufs=4, space="PSUM") as upsum,
            tc.tile_pool(name="upool", bufs=3) as upool,
            tc.tile_pool(name="ppool", bufs=3) as ppool,
        ):
            mult = mybir.AluOpType.mult
            ntile = [0]  # exp tiles consumed, in emission order

            def u_chunk(ps, s, h, n4, slots):
                nsl = slice(h * 1024 + n4 * 512, h * 1024 + (n4 + 1) * 512)
                psl = slice(n4 * 512, (n4 + 1) * 512)
                # 3-slot hi/lo product: hi*hi + lo*hi + hi*lo
                for pi in slots:
                    la, rb = ((0, 0), (2, 0), (0, 2))[pi]
                    nc.tensor.matmul(
                        ps[:, psl],
                        lhsT=lhsU[:, la : la + 2, s * P : (s + 1) * P],
                        rhs=rhsU[:, rb : rb + 2, nsl],
                        perf_mode=DR,
                        start=(pi == 0),
                        stop=(pi == 2),
                    )

            def u_exp(ps, s, h):
                dst = Spair[s // 2][:, s % 2, h * 1024 : (h + 1) * 1024]
                kind = ASSIGN[ntile[0]]
                ntile[0] += 1
                if kind == "A":
                    nc.scalar.activation(
                        dst, ps[:], Exp, bias=ebias[:], scale=LN2,
                    )
                else:  # 'D'/'Y': DVE pass 1; pass 2 on DVE or Pool
                    pool, tag = (upool, "ui") if kind == "D" else (ppool, "pui")
                    ui = pool.tile([P, SEQ // 2], i32, tag=tag)
                    nc.vector.tensor_scalar_mul(ui[:], ps[:], float(2.0**23))
                    eng2 = nc.vector if kind == "D" else nc.gpsimd
                    eng2.tensor_scalar(
                        dst, ui.bitcast(f32)[:], 1.0, 0.0, mult, add,
                    )

            def u_transpose(s):
                nc.sync.dma_start_transpose(
                    Tt[s].bitcast(u16).rearrange("p a b c -> p a (b c)"),
                    Spair[s // 2].bitcast(u16)[:, s % 2, :],
                )

            # Strips 0-7 run CHUNK-major, following the input-DMA arrival
            # order, so the PE does every matmul the moment its operands
            # land instead of head-of-line blocking on a strip's last chunk
            # (saves ~2us of PE idle during the ~6.5us input stream).  Only
            # THREE strips open simultaneously: strip 3's h0 then lands on
            # the ring's never-used 4th slot, bridging the wait for the
            # first three exps to release their banks.
            ps03 = [
                upsum.tile([P, SEQ // 2], f32, tag="ups", name=f"ups0{k}")
                for k in range(3)
            ]
            for s in range(3):  # rhs hi c0 + lhs c0
                u_chunk(ps03[s], s, 0, 0, (0, 1))
            for s in range(3):  # + rhs lo c0
                u_chunk(ps03[s], s, 0, 0, (2,))
            for s in range(3):  # + rhs c1 completes h0
                u_chunk(ps03[s], s, 0, 1, (0, 1, 2))
                u_exp(ps03[s], s, 0)
            for s in range(3, 8):  # + lhs c1: strips 3-7 h0
                ps = upsum.tile([P, SEQ // 2], f32, tag="ups")
                u_chunk(ps, s, 0, 0, (0, 1, 2))
                u_chunk(ps, s, 0, 1, (0, 1, 2))
                u_exp(ps, s, 0)
            for s in range(8):  # + rhs c2/c3: strips 0-7 h1
                ps = upsum.tile([P, SEQ // 2], f32, tag="ups")
                u_chunk(ps, s, 1, 0, (0, 1, 2))
                u_chunk(ps, s, 1, 1, (0, 1, 2))
                u_exp(ps, s, 1)
                u_transpose(s)
            # Strips 8-15: everything is resident by now; strip-major.
            for s in range(8, NS):
                for h in range(2):
                    ps = upsum.tile([P, SEQ // 2], f32, tag="ups")
                    u_chunk(ps, s, h, 0, (0, 1, 2))
                    u_chunk(ps, s, h, 1, (0, 1, 2))
                    u_exp(ps, s, h)
                u_transpose(s)

        Recip = mybir.ActivationFunctionType.Reciprocal
        with (
            tc.tile_pool(name="q1ps", bufs=3, space="PSUM") as q1ps,
            tc.tile_pool(name="p12ps", bufs=3, space="PSUM") as p12ps,
            tc.tile_pool(name="npsum", bufs=2, space="PSUM") as npsum,
            tc.tile_pool(name="tmp", bufs=6) as tmp,
            tc.tile_pool(name="gpool", bufs=6) as gpool,
        ):
            # ---- Q1 + finalize: host pre-scales the delta column by 1/QS
            # so dividing by (norm/QS) scales q1g by QS directly ----
            for J in range(NS):
                c0 = 256 * J if J < 8 else 256 * (J - 8) + 1
                ps = q1ps.tile([P, 257], f32, tag="q1")
                # contraction over i accumulates ADJACENT strip pairs so each
                # pass depends on exactly one Spair tile (order is free)
                for kp in range(8):
                    nc.tensor.matmul(
                        ps[:],
                        lhsT=Spair[kp][:, :, c0 : c0 + 255 : 2],
                        rhs=ownd_sb[:, 2 * kp : 2 * kp + 2, 0:257],
                        perf_mode=DR,
                        start=(kp == 0),
                        stop=(kp == 7),
                    )
                # DVE reciprocal, then alternate the scale between DVE and
                # Act so the Q1 finalize doesn't serialize on one engine
                # (GPSIMD can't read the psum; hw tensor_scalar has no divide)
                scq = tmp.tile([P, 1], f32, tag="scq")
                nc.vector.reciprocal(scq[:], ps[:, 256:257])
                if J % 2 == 0:
                    nc.vector.tensor_scalar_mul(
                        comb[:, J, 256:512], ps[:, 0:256], scq[:]
                    )
                else:
                    nc.scalar.activation(
                        comb[:, J, 256:512], ps[:, 0:256], Copy, scale=scq[:]
                    )

            # ---- P: one 512-wide DRSwInterleave matmul per (J, st), plus an
            # N=1 matmul on the comb ones-column (col 512) accumulating the
            # A-softmax normalizer in the same (SwInterleave-reversed) row
            # order as the numerators, so the per-row reciprocal scale needs
            # no reversal fixup ----
            gview = g.rearrange("(G t p) c -> p G t c", p=P, t=2)
            for grp in range(NS // 2):
                gt = gpool.tile([P, 2, 2 * H], bf16, tag="gt")
                for t in range(2):
                    J = grp * 2 + t
                    ps12 = p12ps.tile([P, 512], f32, tag="p12")
                    # rotating per-J normalizer psum tile (a single shared
                    # tile would stall the next group's first matmul on the
                    # reciprocal's read)
                    psn = npsum.tile([P, 1], f32, tag="psn")
                    last = grp == NS // 2 - 1 and t == 1
                    if last:
                        # all normalizer matmuls first: the reciprocal runs
                        # while the numerator matmuls are still streaming
                        for st in range(NSQ):
                            nc.tensor.matmul(
                                psn[:], lhsT=Tt[J][:, st, :, :],
                                rhs=comb[:, st : st + 9 : 8, 512:513],
                                perf_mode=DRS,
                                start=(st == 0), stop=(st == 7),
                            )
                    # A1 columns (0:256, plain oth rhs — independent of the
                    # Q1 finalize) run first so the P phase enters without
                    # waiting on the last q1g scales; the A2 columns
                    # (256:512, q1g rhs) follow once those land.  Same PE
                    # cycles as one 512-wide matmul per (J, st).
                    for st in range(NSQ):
                        # forward pair-adjacent view; hw SwInterleave decode
                        # reverses columns, so psum partition j holds row
                        # i = J*128 + 127 - j (un-reversed on the host)
                        lv = Tt[J][:, st, :, :]
                        nc.tensor.matmul(
                            ps12[:, 0:256],
                            lhsT=lv,
                            rhs=comb[:, st : st + 9 : 8, 0:256],
                            perf_mode=DRS,
                            start=(st == 0),
                            stop=(st == 7),
                        )
                        if not last:
                            nc.tensor.matmul(
                                psn[:], lhsT=lv,
                                rhs=comb[:, st : st + 9 : 8, 512:513],
                                perf_mode=DRS,
                                start=(st == 0), stop=(st == 7),
                            )
                    for st in range(NSQ):
                        nc.tensor.matmul(
                            ps12[:, 256:512],
                            lhsT=Tt[J][:, st, :, :],
                            rhs=comb[:, st : st + 9 : 8, 256:512],
                            perf_mode=DRS,
                            start=(st == 0),
                            stop=(st == 7),
                        )
                    sc12 = tmp.tile([P, 1], f32, tag="sc12")
                    nc.vector.reciprocal(sc12[:], psn[:])
                    if t == 0:
                        nc.vector.tensor_scalar_mul(gt[:, t, :], ps12[:], sc12[:])
                        if grp == NS // 2 - 1:
                            # issue J14's output immediately so its DMA's
                            # HWDGE slot precedes the final small piece
                            nc.scalar.dma_start(gview[:, grp, 0], gt[:, 0])
                    elif not last:
                        nc.scalar.activation(
                            gt[:, t, :], ps12[:], Copy, scale=sc12[:]
                        )
                    else:
                        # final tile: one full-width DVE scale into its own
                        # tile, one small DMA right behind it (split halves
                        # would serialize anyway on the tile's write order)
                        gtc = tmp.tile([P, 512], bf16, tag="gtc")
                        nc.vector.tensor_scalar_mul(gtc[:], ps12[:], sc12[:])
                        nc.sync.dma_start(gview[:, grp, 1], gtc[:])
                # alternate output DMAs between the Act and SP queues: one
                # queue's issue serialization (~700-900ns) starves gt buffers
                if grp < NS // 2 - 1:
                    eng = nc.scalar if grp % 2 == 0 else nc.sync
                    eng.dma_start(gview[:, grp], gt[:])


def build_nc(reps=1):
    nc = bacc.Bacc(
        "TRN2", target_bir_lowering=False, debug=False, num_devices=N_CORES
    )
    u_lhs = nc.dram_tensor("u_lhs", [P, 4 * SEQ], fp8, kind="ExternalInput").ap()
    u_rhs = nc.dram_tensor("u_rhs", [P, 4 * SEQ], fp8, kind="ExternalInput").ap()
    ownd = nc.dram_tensor("ownd", [P, NS * RC], fp8, kind="ExternalInput").ap()
    comb0 = nc.dram_tensor("comb0", [P, NS * RW], fp8, kind="ExternalInput").ap()
    g = nc.dram_tensor("g", [SEQ, 2 * H], bf16, kind="ExternalOutput").ap()
    with tile.TileContext(nc) as tc:
        for _ in range(reps):
            emit_kernel(nc, tc, u_lhs, u_rhs, ownd, comb0, g)
    nc.compile()
    return nc


def _pmajor(x, inner):
    """[K*P, C] -> [P, K*C] with partition-major swizzle for direct DMA."""
    kp, c = x.shape
    k = kp // inner
    return np.ascontiguousarray(
        x.reshape(k, inner, c).transpose(1, 0, 2).reshape(inner, k * c)
    )


_SIGMA = np.empty(SEQ, np.int64)
_SIGMA[0::2] = np.arange(0, SEQ // 2)
_SIGMA[1::2] = np.arange(SEQ // 2, SEQ)

_REV_EYE = np.ascontiguousarray(np.eye(P, dtype=np.float32)[:, ::-1])


def make_core_inputs(own, oth, w_own, w_oth, w_m, own_mask, oth_mask):
    """Host-side prep of one core's tensors (all small [2048,256]-ish work)."""
    own = np.asarray(own, np.float32)
    oth = np.asarray(oth, np.float32)
    own_bias = np.where(own_mask < 0.5, np.float32(-1e9), np.float32(0.0))
    oth_bias = np.where(oth_mask < 0.5, np.float32(-1e9), np.float32(0.0))
    delta = np.exp(own @ w_own + own_bias).astype(np.float32)
    ln_gamma = (oth @ w_oth + oth_bias).astype(np.float32)
    ln_gamma = np.maximum(ln_gamma, -70.0)

    # steal the least-|w_m| h slot for the ln(gamma) rank-1 injection;
    # balance the remaining rows (a_h ~ w_m[h], b_h ~ 1) into sqrt|w_m[h]|
    # scale on both sides so fp8 hi+lo stays out of the denormal floor
    hstar = int(np.argmin(np.abs(w_m)))
    s_h = np.sqrt(np.maximum(np.abs(w_m), 1e-8)).astype(np.float32)
    s_h[hstar] = 1.0
    # log2-domain: psum must produce log2-logits + YOFF, so scale the
    # own side by log2(e) and put lnG + YOFF*ln2 in the stolen slot
    ownm = own * (w_m / s_h)[None, :] * np.float32(LOG2E)
    ownm[:, hstar] = LOG2E
    oth_c = oth[_SIGMA] * s_h[None, :]
    oth_c[:, hstar] = ln_gamma[_SIGMA] + np.float32(YOFF * LN2)

    def _hilo(mat_t):  # [H, SEQ] f32 -> [2H, SEQ] fp8 rows [hi; lo]
        hi = mat_t.astype(fp8_np)
        lo = (mat_t - hi.astype(np.float32)).astype(fp8_np)
        return np.concatenate([hi, lo], axis=0)

    ownd = np.zeros((SEQ, RC), np.float32)
    ownd[:, :H] = own * delta[:, None]
    ownd[:, H] = delta * (1.0 / QS)  # so 1/norm' = QS/norm = q1g scale
    comb0f = np.zeros((SEQ, RW), np.float32)
    comb0f[:, :H] = oth
    comb0f[:, 2 * H] = 1.0  # ones-column: P psum col 512 = A-softmax norm

    return {
        "u_lhs": _pmajor(_hilo(np.ascontiguousarray(ownm.T)), P),
        "u_rhs": _pmajor(_hilo(np.ascontiguousarray(oth_c.T)), P),
        "ownd": _pmajor(ownd, P).astype(fp8_np),
        "comb0": _pmajor(comb0f, P).astype(fp8_np),
    }


def make_all_inputs(encode_input1, encode_input2, input1_mask, input2_mask, W):
    E_q = np.asarray(encode_input1, np.float32)  # [B, SQ, H]
    E_p = np.asarray(encode_input2, np.float32)  # [B, SP, H]
    m1 = np.asarray(input1_mask, np.float32)  # [B, SP] masks p
    m2 = np.asarray(input2_mask, np.float32)  # [B, SQ] masks q
    W = np.asarray(W, np.float32)
    w_q, w_p, w_m = W[:H], W[H : 2 * H], W[2 * H :]

    in_maps = []
    for c in range(N_CORES):
        b, side = c // 2, c % 2
        if side == 0:  # produces G_q_p[b] (p-indexed)
            in_maps.append(
                make_core_inputs(E_p[b], E_q[b], w_p, w_q, w_m, m1[b], m2[b])
            )
        else:  # produces G_p_q[b] (q-indexed)
            in_maps.append(
                make_core_inputs(E_q[b], E_p[b], w_q, w_p, w_m, m2[b], m1[b])
            )
    return in_maps


def assemble_output(own_f32, g_bf16):
    """G = [E, A1, A2, E*A1, E*A2] f32 from device [A1|32*A2] bf16.

    Device rows come out reversed within each 128-block (SwInterleave
    column reversal); un-reverse here."""
    g_bf16 = np.asarray(g_bf16).reshape(NS, P, 2 * H)[:, ::-1, :].reshape(
        SEQ, 2 * H
    )
    a1 = np.asarray(g_bf16[:, :H], np.float32)
    a2 = np.asarray(g_bf16[:, H:], np.float32) * np.float32(1.0 / QS)
    return np.concatenate(
        [own_f32, a1, a2, own_f32 * a1, own_f32 * a2], axis=-1
    )


_NC_CACHE = {}


def get_nc():
    if "nc" not in _NC_CACHE:
        _NC_CACHE["nc"] = build_nc()
    return _NC_CACHE["nc"]


def kernel(encode_input1, encode_input2, input1_mask, input2_mask, W):
    nc = get_nc()
    E_q = np.asarray(encode_input1, np.float32)
    E_p = np.asarray(encode_input2, np.float32)
    in_maps = make_all_inputs(
        encode_input1, encode_input2, input1_mask, input2_mask, W
    )
    res = run_bass_kernel_spmd(nc, in_maps, list(range(N_CORES)))
    G_q_p = np.stack(
        [assemble_output(E_p[b], res.results[2 * b]["g"]) for b in range(B)]
    )
    G_p_q = np.stack(
        [assemble_output(E_q[b], res.results[2 * b + 1]["g"]) for b in range(B)]
    )
    return (G_p_q, G_q_p)


if __name__ == "__main__":
    # CoreSim numerics self-check of one core against numpy.
    from concourse.bass_interp import CoreSim

    rng = np.random.default_rng(0)
    own = rng.standard_normal((SEQ, H)).astype(np.float32)
    oth = rng.standard_normal((SEQ, H)).astype(np.float32)
    Wv = (rng.standard_normal(3 * H) / np.sqrt(3 * H)).astype(np.float32)
    w_own, w_oth, w_m = Wv[:H], Wv[H : 2 * H], Wv[2 * H :]
    ones = np.ones(SEQ, np.float32)

    nc = bacc.Bacc("TRN2", target_bir_lowering=False, debug=False, num_devices=1)
    u_lhs = nc.dram_tensor("u_lhs", [P, 4 * SEQ], fp8, kind="ExternalInput").ap()
    u_rhs = nc.dram_tensor("u_rhs", [P, 4 * SEQ], fp8, kind="ExternalInput").ap()
    ownd = nc.dram_tensor("ownd", [P, NS * RC], fp8, kind="ExternalInput").ap()
    comb0 = nc.dram_tensor("comb0", [P, NS * RW], fp8, kind="ExternalInput").ap()
    g = nc.dram_tensor("g", [SEQ, 2 * H], bf16, kind="ExternalOutput").ap()
    with tile.TileContext(nc) as tc:
        emit_kernel(nc, tc, u_lhs, u_rhs, ownd, comb0, g)
    nc.compile()
    print("compiled")

    ins = make_core_inputs(own, oth, w_own, w_oth, w_m, ones, ones)
    sim = CoreSim(nc, require_finite=False, require_nnan=False)
    for k, v in ins.items():
        sim.tensor(k)[:] = v
    sim.simulate(check_with_hw=False)
    got = np.asarray(sim.tensor("g")).astype(np.float32)
    got = got.reshape(NS, P, 2 * H)[:, ::-1, :].reshape(SEQ, 2 * H)
    got[:, H:] *= 1.0 / QS

    # numpy reference for this core's side
    delta = np.exp(own @ w_own)
    gamma = np.exp(oth @ w_oth)
    Sref = np.exp((own * w_m) @ oth.T)  # [i, q]
    A_w = Sref * gamma[None, :]  # A-softmax numer weights over q
    A1 = (A_w @ oth) / A_w.sum(1, keepdims=True)
    B_w = Sref * delta[:, None]  # B-softmax weights over i
    B1 = (B_w.T @ own) / B_w.sum(0)[:, None]  # [q, H]
    A2 = (A_w @ B1) / A_w.sum(1, keepdims=True)
    want = np.concatenate([A1, A2], axis=-1)
    err = np.abs(got - want)
    scale = np.abs(want).max()
    print(f"A1A2: absmax={err.max():.3e} scale={scale:.3f} rel={err.max()/scale:.3e}")

    # full-output check
    got_a1, got_a2 = got[:, :H], got[:, H:]
    G_got = np.concatenate([own, got_a1, got_a2, own * got_a1, own * got_a2], -1)
    G_want = np.concatenate([own, A1, A2, own * A1, own * A2], -1)
    gerr = np.abs(G_got - G_want)
    gscale = np.abs(G_want).max()
    print(f"G: absmax={gerr.max():.3e} scale={gscale:.2f} rel={gerr.max()/gscale:.3e}")



# revision 94
# speedup vs baseline: 1.0024x; 1.0024x over previous
"""Trainium2 Bass kernel for the dual-softmax interaction (BiDAF-style) layer.

fp8-e4m3 DoubleRow attention products + fp8 hi/lo split-K logits.

Key tricks vs the bf16 baseline:
- Logit matmul runs in fp8 DoubleRow with a 3-slot hi/lo residual
  decomposition (hi*hi + lo*hi + hi*lo, per-h sqrt|w_m| balanced), ~0.01
  worst-case logit error at 0.75x the fp16 PE cost.
- The gamma (=exp(sq)) column-scaling is folded INTO S by stealing the
  h-slot with the smallest |w_m|: that lhsT row becomes ones and the rhs
  row ln(gamma), so S''[i,c] = gamma_c*exp(tri) comes out of the exp
  directly.  This removes the gamma column from the attention rhs and
  merges P1|P2 into ONE 512-wide DoubleRow matmul.
- Both softmax normalizers come out of matmuls: the B-side as ownd's
  delta column (Q1 psum col 256), the A-side as comb's ones-column via a
  tiny N=1 matmul chasing each P matmul — psum rows and normalizer share
  the SwInterleave row order, so no reversal fixup anywhere.
- The exp of the 2048x2048 logit matrix is spread over THREE engines
  (Act exp / DVE 2-pass Schraudolph / DVE pass 1 + Pool pass 2), sized
  so every engine stays just below the PE's 20.5us of matmul work.
  GPSIMD cannot touch PSUM on real hw, so Pool only sees the SBUF i32
  intermediate; hw tensor_scalar has no divide, so scales go through
  DVE reciprocal + multiply.
- PE p-state warmup matmuls burn the cost model's 3us clock ramp while
  the first input DMAs are in flight; input DMAs stream in need-by
  order (HWDGE is a serial ~630ns/DMA device, transfers ~2.84us/MB).
- S'' is stored fp8 with columns in sigma order (c=2u -> q=u,
  c=2u+1 -> q=1024+u); the uint16 pair view of S'' is DMA-transposed so
  T lands as [p, st, i, r] with q = st*128 + p + 1024r.  The P matmuls
  consume T via MatmulPerfMode.DoubleRowSwInterleave (pairs adjacent,
  columns reversed = exactly the SwInterleave weight layout); rhs pairs
  are comb strip pairs (st,st+8).  S/T live in per-strip(-pair) tiles.

Math per core (own/oth swapped between the two cores of a batch):
    S''[i,c]  = gamma_c exp(sum_h (own*w_m)[i,h] oth[sigma(c),h])
    Q1[q,:]   = S''^T @ [own*delta | delta]  -> gamma_q*(B1 numer|norm)
                (gamma_q cancels in the ratio; q1g = 32*B1_normalized)
    P[i,:]    = T''^T @ [oth | q1g | 1]      -> A1|32*A2 numers | A-norm
Device returns [A1 | 32*A2] bf16; host assembles G = [E, A1, A2, E*A1,
E*A2] f32 from the original f32 inputs.
"""

import sys

if "/opt/trn_rl_repo" not in sys.path:
    sys.path.insert(0, "/opt/trn_rl_repo")

import numpy as np

import concourse.bass as bass  # noqa: F401  (registers AP machinery)
import concourse.mybir as mybir
import concourse.tile as tile
from concourse import bacc
from concourse.bass_utils import run_bass_kernel_spmd

B, SEQ, H = 4, 2048, 256
P = 128
NS = SEQ // P  # 16 i-strips
NSQ = NS // 2  # 8 q-pair strips
RC = 260  # ownd row: 256 delta*own + 1 delta + 3 pad
RW = 520  # comb row: 256 oth + 256 q1g + 1 ones (A-norm) + 7 pad
QA = 260  # qacc row: 257 used + 3 pad
QS = 32.0  # q1g pre-scale keeping B1 values in fp8 normal range
LOG2E = float(np.log2(np.e))
LN2 = float(np.log(2.0))
SB = 0.0435  # Schraudolph bias (truncating f32->i32 convert)
YOFF = 127.0 - SB  # folded into the ln(gamma) row; psum = log2-logits + YOFF
N_CORES = 8

f32 = mybir.dt.float32
bf16 = mybir.dt.bfloat16
fp8 = mybir.dt.float8e4
u16 = mybir.dt.uint16
i32 = mybir.dt.int32
fp8_np = mybir.dt.np(fp8)
bf16_np = mybir.dt.np(bf16)
DR = mybir.MatmulPerfMode.DoubleRow
DRS = mybir.MatmulPerfMode.DoubleRowSwInterleave

def emit_kernel(nc, tc, u_lhs, u_rhs, ownd, comb0, g):
    Exp = mybir.ActivationFunctionType.Exp
    Copy = mybir.ActivationFunctionType.Copy
    add = mybir.AluOpType.add

    with (
        tc.tile_pool(name="big", bufs=1) as big,
        tc.tile_pool(name="small", bufs=1) as small,
    ):
        # S and T are split into per-strip(-pair) tiles so downstream reads
        # depend only on the strips they touch, not on the whole U phase
        # (a single big tile made Q1's first matmul wait for the LAST exp).
        Spair = [
            big.tile([P, 2, SEQ], fp8, name=f"Spair{k}") for k in range(NS // 2)
        ]
        Tt = [big.tile([P, NSQ, P, 2], fp8, name=f"Tt{s}") for s in range(NS)]
        # comb is THREE tiles so the P-phase A1 matmuls (oth) don't pick
        # up a coarse-tile dependency on the Q1-finalize q1g writes
        comb_oth = big.tile([P, NS, H], fp8)  # strip k: rows q=k*128+p
        comb_q1g = big.tile([P, NS, H], fp8)  # written by Q1 finalize
        comb_one = big.tile([P, NS, 8], fp8)  # ones: A-norm matmul rhs
        ownd_sb = big.tile([P, NS, RC], fp8)
        # U operands in fp8 hi+lo split-K: k-tiles [k0hi, k1hi, k0lo, k1lo]
        lhsU = big.tile([P, 4, SEQ], fp8)
        rhsU = big.tile([P, 4, SEQ], fp8)
        ebias = small.tile([P, 1], f32)  # -YOFF*ln2 activation bias
        nc.vector.memset(ebias[:], -YOFF * LN2)

        # PE p-state warmup: the cost model runs matmuls at half speed until
        # the PE has been continuously busy for 3us.  Burn that ramp on dummy
        # bf16 matmuls over a memset tile while the first input DMAs are in
        # flight, so every real matmul runs at full clock.  The memset runs
        # on the Pool engine, whose queue is free right after the preamble.
        wz = small.tile([P, 512], bf16)
        nc.gpsimd.memset(wz[:], 0.0)
        with tc.tile_pool(name="warm", bufs=1, space="PSUM") as warm:
            wps = warm.tile([P, 512], f32, tag="warm")
            for _ in range(5):
                nc.tensor.matmul(
                    wps[:], lhsT=wz[:, 0:P], rhs=wz[:], start=True, stop=True
                )
            # consume so the pool close doesn't warn about unread psum
            nc.vector.tensor_scalar_mul(wz[:, 0:1], wps[:, 0:1], 0.0)

        # Everything streams through SP/HWDGE (serial, ~630ns/DMA descriptor
        # gen, wire ~2.84us/MB) in the order the chunk-major U phase below
        # consumes it: rhs hi c0, lhs c0, rhs lo c0, then alternating
        # rhs/lhs columns, and last the Q1/P operands.
        uview = lambda ap: ap.rearrange("p (a b) -> p a b", a=4)
        nc.sync.dma_start(rhsU[:, 0:2, 0:512], uview(u_rhs)[:, 0:2, 0:512])
        nc.sync.dma_start(lhsU[:, :, 0:512], uview(u_lhs)[:, :, 0:512])
        nc.sync.dma_start(rhsU[:, 2:4, 0:512], uview(u_rhs)[:, 2:4, 0:512])
        for c4 in range(1, 4):
            csl = slice(c4 * 512, (c4 + 1) * 512)
            nc.sync.dma_start(rhsU[:, :, csl], uview(u_rhs)[:, :, csl])
            nc.sync.dma_start(lhsU[:, :, csl], uview(u_lhs)[:, :, csl])
        nc.sync.dma_start(ownd_sb.rearrange("p s c -> p (s c)"), ownd[:])
        nc.sync.dma_start(
            comb_oth[:],
            comb0.rearrange("p (s c) -> p s c", c=RW)[:, :, 0:H],
        )
        nc.gpsimd.memset(comb_one[:], 1.0)



        # ---- U phase: S'' = gamma*exp(tri) fp8; T = pair-T ----
        # psum holds y' = log2(S'') + YOFF.  Exp spreads over three engines:
        # 'A' tiles go to the Act engine (Exp with scale=ln2, bias=-YOFF*ln2),
        # 'D' tiles to the DVE (2-pass Schraudolph: truncating f32->i32 of
        # y'*2^23, then a bitcast-f32 copy to fp8), and 'Y' tiles run pass 1
        # on the DVE but hand pass 2 (SBUF->SBUF) to the Pool engine.  GPSIMD
        # cannot touch PSUM on real hw, so Pool can only ever see the i32
        # intermediate.  No row-sum accumulation here: the A-softmax
        # normalizer comes out of the P matmul's ones-column (comb col 512).
        import os as _os
        _odd = list("YYDYYYAYYDYYYAYY")  # odd slots: Y12 D2 A2 (swept best)
        _dflt = "".join(c for pair in zip("A" * 16, _odd) for c in pair)
        ASSIGN = list(_os.environ.get("KASSIGN", _dflt))[:2 * NS]
        with (
            tc.tile_pool(name="upsum", bufs=4, space="PSUM") as upsum,
            tc.tile_pool(name="upool", bufs=3) as upool,
            tc.tile_pool(name="ppool", bufs=3) as ppool,
        ):
            mult = mybir.AluOpType.mult
            ntile = [0]  # exp tiles consumed, in emission order

            def u_chunk(ps, s, h, n4, slots):
                nsl = slice(h * 1024 + n4 * 512, h * 1024 + (n4 + 1) * 512)
                psl = slice(n4 * 512, (n4 + 1) * 512)
                # 3-slot hi/lo product: hi*hi + lo*hi + hi*lo
                for pi in slots:
                    la, rb = ((0, 0), (2, 0), (0, 2))[pi]
                    nc.tensor.matmul(
                        ps[:, psl],
                        lhsT=lhsU[:, la : la + 2, s * P : (s + 1) * P],
                        rhs=rhsU[:, rb : rb + 2, nsl],
                        perf_mode=DR,
                        start=(pi == 0),
                        stop=(pi == 2),
                    )

            def u_exp(ps, s, h):
                dst = Spair[s // 2][:, s % 2, h * 1024 : (h + 1) * 1024]
                kind = ASSIGN[ntile[0]]
                ntile[0] += 1
                if kind == "A":
                    nc.scalar.activation(
                        dst, ps[:], Exp, bias=ebias[:], scale=LN2,
                    )
                else:  # 'D'/'Y': DVE pass 1; pass 2 on DVE or Pool
                    pool, tag = (upool, "ui") if kind == "D" else (ppool, "pui")
                    ui = pool.tile([P, SEQ // 2], i32, tag=tag)
                    nc.vector.tensor_scalar_mul(ui[:], ps[:], float(2.0**23))
                    eng2 = nc.vector if kind == "D" else nc.gpsimd
                    eng2.tensor_scalar(
                        dst, ui.bitcast(f32)[:], 1.0, 0.0, mult, add,
                    )

            def u_transpose(s):
                nc.sync.dma_start_transpose(
                    Tt[s].bitcast(u16).rearrange("p a b c -> p a (b c)"),
                    Spair[s // 2].bitcast(u16)[:, s % 2, :],
                )

            # Strips 0-7 run CHUNK-major, following the input-DMA arrival
            # order, so the PE does every matmul the moment its operands
            # land instead of head-of-line blocking on a strip's last chunk
            # (saves ~2us of PE idle during the ~6.5us input stream).  Only
            # THREE strips open simultaneously: strip 3's h0 then lands on
            # the ring's never-used 4th slot, bridging the wait for the
            # first three exps to release their banks.
            ps03 = [
                upsum.tile([P, SEQ // 2], f32, tag="ups", name=f"ups0{k}")
                for k in range(3)
            ]
            for s in range(3):  # rhs hi c0 + lhs c0
                u_chunk(ps03[s], s, 0, 0, (0, 1))
            for s in range(3):  # + rhs lo c0
                u_chunk(ps03[s], s, 0, 0, (2,))
            for s in range(3):  # + rhs c1 completes h0
                u_chunk(ps03[s], s, 0, 1, (0, 1, 2))
                u_exp(ps03[s], s, 0)
            for s in range(3, 8):  # + lhs c1: strips 3-7 h0
                ps = upsum.tile([P, SEQ // 2], f32, tag="ups")
                u_chunk(ps, s, 0, 0, (0, 1, 2))
                u_chunk(ps, s, 0, 1, (0, 1, 2))
                u_exp(ps, s, 0)
            for s in range(8):  # + rhs c2/c3: strips 0-7 h1
                ps = upsum.tile([P, SEQ // 2], f32, tag="ups")
                u_chunk(ps, s, 1, 0, (0, 1, 2))
                u_chunk(ps, s, 1, 1, (0, 1, 2))
                u_exp(ps, s, 1)
                u_transpose(s)
            # Strips 8-15: everything is resident by now; strip-major.
            for s in range(8, NS):
                for h in range(2):
                    ps = upsum.tile([P, SEQ // 2], f32, tag="ups")
                    u_chunk(ps, s, h, 0, (0, 1, 2))
                    u_chunk(ps, s, h, 1, (0, 1, 2))
                    u_exp(ps, s, h)
                u_transpose(s)

        Recip = mybir.ActivationFunctionType.Reciprocal
        with (
            tc.tile_pool(name="q1ps", bufs=3, space="PSUM") as q1ps,
            tc.tile_pool(name="p12ps", bufs=3, space="PSUM") as p12ps,
            tc.tile_pool(name="npsum", bufs=2, space="PSUM") as npsum,
            tc.tile_pool(name="tmp", bufs=6) as tmp,
            tc.tile_pool(name="gpool", bufs=6) as gpool,
        ):
            # ---- Q1 + finalize: host pre-scales the delta column by 1/QS
            # so dividing by (norm/QS) scales q1g by QS directly ----
            for J in range(NS):
                c0 = 256 * J if J < 8 else 256 * (J - 8) + 1
                ps = q1ps.tile([P, 257], f32, tag="q1")
                # contraction over i accumulates ADJACENT strip pairs so each
                # pass depends on exactly one Spair tile (order is free)
                for kp in range(8):
                    nc.tensor.matmul(
                        ps[:],
                        lhsT=Spair[kp][:, :, c0 : c0 + 255 : 2],
                        rhs=ownd_sb[:, 2 * kp : 2 * kp + 2, 0:257],
                        perf_mode=DR,
                        start=(kp == 0),
                        stop=(kp == 7),
                    )
                # DVE reciprocal, then alternate the scale between DVE and
                # Act so the Q1 finalize doesn't serialize on one engine
                # (GPSIMD can't read the psum; hw tensor_scalar has no divide)
                scq = tmp.tile([P, 1], f32, tag="scq")
                nc.vector.reciprocal(scq[:], ps[:, 256:257])
                if J % 2 == 0:
                    nc.vector.tensor_scalar_mul(
                        comb_q1g[:, J, :], ps[:, 0:256], scq[:]
                    )
                else:
                    nc.scalar.activation(
                        comb_q1g[:, J, :], ps[:, 0:256], Copy, scale=scq[:]
                    )

            # ---- P: one 512-wide DRSwInterleave matmul per (J, st), plus an
            # N=1 matmul on the comb ones-column (col 512) accumulating the
            # A-softmax normalizer in the same (SwInterleave-reversed) row
            # order as the numerators, so the per-row reciprocal scale needs
            # no reversal fixup ----
            gview = g.rearrange("(G t p) c -> p G t c", p=P, t=2)
            for grp in range(NS // 2):
                gt = gpool.tile([P, 2, 2 * H], bf16, tag="gt")
                for t in range(2):
                    J = grp * 2 + t
                    ps12 = p12ps.tile([P, 512], f32, tag="p12")
                    # rotating per-J normalizer psum tile (a single shared
                    # tile would stall the next group's first matmul on the
                    # reciprocal's read)
                    psn = npsum.tile([P, 1], f32, tag="psn")
                    last = grp == NS // 2 - 1 and t == 1
                    if last:
                        # all normalizer matmuls first: the reciprocal runs
                        # while the numerator matmuls are still streaming
                        for st in range(NSQ):
                            nc.tensor.matmul(
                                psn[:], lhsT=Tt[J][:, st, :, :],
                                rhs=comb_one[:, st : st + 9 : 8, 0:1],
                                perf_mode=DRS,
                                start=(st == 0), stop=(st == 7),
                            )
                    # A1 columns (0:256, plain oth rhs — independent of the
                    # Q1 finalize) run first so the P phase enters without
                    # waiting on the last q1g scales; the A2 columns
                    # (256:512, q1g rhs) follow once those land.  Same PE
                    # cycles as one 512-wide matmul per (J, st).
                    for st in range(NSQ):
                        # forward pair-adjacent view; hw SwInterleave decode
                        # reverses columns, so psum partition j holds row
                        # i = J*128 + 127 - j (un-reversed on the host)
                        lv = Tt[J][:, st, :, :]
                        nc.tensor.matmul(
                            ps12[:, 0:256],
                            lhsT=lv,
                            rhs=comb_oth[:, st : st + 9 : 8, :],
                            perf_mode=DRS,
                            start=(st == 0),
                            stop=(st == 7),
                        )
                        if not last:
                            nc.tensor.matmul(
                                psn[:], lhsT=lv,
                                rhs=comb_one[:, st : st + 9 : 8, 0:1],
                                perf_mode=DRS,
                                start=(st == 0), stop=(st == 7),
                            )
                    for st in range(NSQ):
                        nc.tensor.matmul(
                            ps12[:, 256:512],
                            lhsT=Tt[J][:, st, :, :],
                            rhs=comb_q1g[:, st : st + 9 : 8, :],
                            perf_mode=DRS,
                            start=(st == 0),
                            stop=(st == 7),
                        )
                    sc12 = tmp.tile([P, 1], f32, tag="sc12")
                    nc.vector.reciprocal(sc12[:], psn[:])
                    if t == 0:
                        nc.vector.tensor_scalar_mul(gt[:, t, :], ps12[:], sc12[:])
                        if grp == NS // 2 - 1:
                            # issue J14's output immediately so its DMA's
                            # HWDGE slot precedes the final small piece
                            nc.scalar.dma_start(gview[:, grp, 0], gt[:, 0])
                    elif not last:
                        nc.scalar.activation(
                            gt[:, t, :], ps12[:], Copy, scale=sc12[:]
                        )
                    else:
                        # final tile: one full-width DVE scale into its own
                        # tile, one small DMA right behind it (split halves
                        # would serialize anyway on the tile's write order)
                        gtc = tmp.tile([P, 512], bf16, tag="gtc")
                        nc.vector.tensor_scalar_mul(gtc[:], ps12[:], sc12[:])
                        nc.sync.dma_start(gview[:, grp, 1], gtc[:])
                # alternate output DMAs between the Act and SP queues: one
                # queue's issue serialization (~700-900ns) starves gt buffers
                if grp < NS // 2 - 1:
                    eng = nc.scalar if grp % 2 == 0 else nc.sync
                    eng.dma_start(gview[:, grp], gt[:])


def build_nc(reps=1):
    nc = bacc.Bacc(
        "TRN2", target_bir_lowering=False, debug=False, num_devices=N_CORES
    )
    u_lhs = nc.dram_tensor("u_lhs", [P, 4 * SEQ], fp8, kind="ExternalInput").ap()
    u_rhs = nc.dram_tensor("u_rhs", [P, 4 * SEQ], fp8, kind="ExternalInput").ap()
    ownd = nc.dram_tensor("ownd", [P, NS * RC], fp8, kind="ExternalInput").ap()
    comb0 = nc.dram_tensor("comb0", [P, NS * RW], fp8, kind="ExternalInput").ap()
    g = nc.dram_tensor("g", [SEQ, 2 * H], bf16, kind="ExternalOutput").ap()
    with tile.TileContext(nc) as tc:
        for _ in range(reps):
            emit_kernel(nc, tc, u_lhs, u_rhs, ownd, comb0, g)
    nc.compile()
    return nc


def _pmajor(x, inner):
    """[K*P, C] -> [P, K*C] with partition-major swizzle for direct DMA."""
    kp, c = x.shape
    k = kp // inner
    return np.ascontiguousarray(
        x.reshape(k, inner, c).transpose(1, 0, 2).reshape(inner, k * c)
    )


_SIGMA = np.empty(SEQ, np.int64)
_SIGMA[0::2] = np.arange(0, SEQ // 2)
_SIGMA[1::2] = np.arange(SEQ // 2, SEQ)

_REV_EYE = np.ascontiguousarray(np.eye(P, dtype=np.float32)[:, ::-1])


def make_core_inputs(own, oth, w_own, w_oth, w_m, own_mask, oth_mask):
    """Host-side prep of one core's tensors (all small [2048,256]-ish work)."""
    own = np.asarray(own, np.float32)
    oth = np.asarray(oth, np.float32)
    own_bias = np.where(own_mask < 0.5, np.float32(-1e9), np.float32(0.0))
    oth_bias = np.where(oth_mask < 0.5, np.float32(-1e9), np.float32(0.0))
    delta = np.exp(own @ w_own + own_bias).astype(np.float32)
    ln_gamma = (oth @ w_oth + oth_bias).astype(np.float32)
    ln_gamma = np.maximum(ln_gamma, -70.0)

    # steal the least-|w_m| h slot for the ln(gamma) rank-1 injection;
    # balance the remaining rows (a_h ~ w_m[h], b_h ~ 1) into sqrt|w_m[h]|
    # scale on both sides so fp8 hi+lo stays out of the denormal floor
    hstar = int(np.argmin(np.abs(w_m)))
    s_h = np.sqrt(np.maximum(np.abs(w_m), 1e-8)).astype(np.float32)
    s_h[hstar] = 1.0
    # log2-domain: psum must produce log2-logits + YOFF, so scale the
    # own side by log2(e) and put lnG + YOFF*ln2 in the stolen slot
    ownm = own * (w_m / s_h)[None, :] * np.float32(LOG2E)
    ownm[:, hstar] = LOG2E
    oth_c = oth[_SIGMA] * s_h[None, :]
    oth_c[:, hstar] = ln_gamma[_SIGMA] + np.float32(YOFF * LN2)

    def _hilo(mat_t):  # [H, SEQ] f32 -> [2H, SEQ] fp8 rows [hi; lo]
        hi = mat_t.astype(fp8_np)
        lo = (mat_t - hi.astype(np.float32)).astype(fp8_np)
        return np.concatenate([hi, lo], axis=0)

    ownd = np.zeros((SEQ, RC), np.float32)
    ownd[:, :H] = own * delta[:, None]
    ownd[:, H] = delta * (1.0 / QS)  # so 1/norm' = QS/norm = q1g scale
    comb0f = np.zeros((SEQ, RW), np.float32)
    comb0f[:, :H] = oth
    comb0f[:, 2 * H] = 1.0  # ones-column: P psum col 512 = A-softmax norm

    return {
        "u_lhs": _pmajor(_hilo(np.ascontiguousarray(ownm.T)), P),
        "u_rhs": _pmajor(_hilo(np.ascontiguousarray(oth_c.T)), P),
        "ownd": _pmajor(ownd, P).astype(fp8_np),
        "comb0": _pmajor(comb0f, P).astype(fp8_np),
    }


def make_all_inputs(encode_input1, encode_input2, input1_mask, input2_mask, W):
    E_q = np.asarray(encode_input1, np.float32)  # [B, SQ, H]
    E_p = np.asarray(encode_input2, np.float32)  # [B, SP, H]
    m1 = np.asarray(input1_mask, np.float32)  # [B, SP] masks p
    m2 = np.asarray(input2_mask, np.float32)  # [B, SQ] masks q
    W = np.asarray(W, np.float32)
    w_q, w_p, w_m = W[:H], W[H : 2 * H], W[2 * H :]

    in_maps = []
    for c in range(N_CORES):
        b, side = c // 2, c % 2
        if side == 0:  # produces G_q_p[b] (p-indexed)
            in_maps.append(
                make_core_inputs(E_p[b], E_q[b], w_p, w_q, w_m, m1[b], m2[b])
            )
        else:  # produces G_p_q[b] (q-indexed)
            in_maps.append(
                make_core_inputs(E_q[b], E_p[b], w_q, w_p, w_m, m2[b], m1[b])
            )
    return in_maps


def assemble_output(own_f32, g_bf16):
    """G = [E, A1, A2, E*A1, E*A2] f32 from device [A1|32*A2] bf16.

    Device rows come out reversed within each 128-block (SwInterleave
    column reversal); un-reverse here."""
    g_bf16 = np.asarray(g_bf16).reshape(NS, P, 2 * H)[:, ::-1, :].reshape(
        SEQ, 2 * H
    )
    a1 = np.asarray(g_bf16[:, :H], np.float32)
    a2 = np.asarray(g_bf16[:, H:], np.float32) * np.float32(1.0 / QS)
    return np.concatenate(
        [own_f32, a1, a2, own_f32 * a1, own_f32 * a2], axis=-1
    )


_NC_CACHE = {}


def get_nc():
    if "nc" not in _NC_CACHE:
        _NC_CACHE["nc"] = build_nc()
    return _NC_CACHE["nc"]


def kernel(encode_input1, encode_input2, input1_mask, input2_mask, W):
    nc = get_nc()
    E_q = np.asarray(encode_input1, np.float32)
    E_p = np.asarray(encode_input2, np.float32)
    in_maps = make_all_inputs(
        encode_input1, encode_input2, input1_mask, input2_mask, W
    )
    res = run_bass_kernel_spmd(nc, in_maps, list(range(N_CORES)))
    G_q_p = np.stack(
        [assemble_output(E_p[b], res.results[2 * b]["g"]) for b in range(B)]
    )
    G_p_q = np.stack(
        [assemble_output(E_q[b], res.results[2 * b + 1]["g"]) for b in range(B)]
    )
    return (G_p_q, G_q_p)


if __name__ == "__main__":
    # CoreSim numerics self-check of one core against numpy.
    from concourse.bass_interp import CoreSim

    rng = np.random.default_rng(0)
    own = rng.standard_normal((SEQ, H)).astype(np.float32)
    oth = rng.standard_normal((SEQ, H)).astype(np.float32)
    Wv = (rng.standard_normal(3 * H) / np.sqrt(3 * H)).astype(np.float32)
    w_own, w_oth, w_m = Wv[:H], Wv[H : 2 * H], Wv[2 * H :]
    ones = np.ones(SEQ, np.float32)

    nc = bacc.Bacc("TRN2", target_bir_lowering=False, debug=False, num_devices=1)
    u_lhs = nc.dram_tensor("u_lhs", [P, 4 * SEQ], fp8, kind="ExternalInput").ap()
    u_rhs = nc.dram_tensor("u_rhs", [P, 4 * SEQ], fp8, kind="ExternalInput").ap()
    ownd = nc.dram_tensor("ownd", [P, NS * RC], fp8, kind="ExternalInput").ap()
    comb0 = nc.dram_tensor("comb0", [P, NS * RW], fp8, kind="ExternalInput").ap()
    g = nc.dram_tensor("g", [SEQ, 2 * H], bf16, kind="ExternalOutput").ap()
    with tile.TileContext(nc) as tc:
        emit_kernel(nc, tc, u_lhs, u_rhs, ownd, comb0, g)
    nc.compile()
    print("compiled")

    ins = make_core_inputs(own, oth, w_own, w_oth, w_m, ones, ones)
    sim = CoreSim(nc, require_finite=False, require_nnan=False)
    for k, v in ins.items():
        sim.tensor(k)[:] = v
    sim.simulate(check_with_hw=False)
    got = np.asarray(sim.tensor("g")).astype(np.float32)
    got = got.reshape(NS, P, 2 * H)[:, ::-1, :].reshape(SEQ, 2 * H)
    got[:, H:] *= 1.0 / QS

    # numpy reference for this core's side
    delta = np.exp(own @ w_own)
    gamma = np.exp(oth @ w_oth)
    Sref = np.exp((own * w_m) @ oth.T)  # [i, q]
    A_w = Sref * gamma[None, :]  # A-softmax numer weights over q
    A1 = (A_w @ oth) / A_w.sum(1, keepdims=True)
    B_w = Sref * delta[:, None]  # B-softmax weights over i
    B1 = (B_w.T @ own) / B_w.sum(0)[:, None]  # [q, H]
    A2 = (A_w @ B1) / A_w.sum(1, keepdims=True)
    want = np.concatenate([A1, A2], axis=-1)
    err = np.abs(got - want)
    scale = np.abs(want).max()
    print(f"A1A2: absmax={err.max():.3e} scale={scale:.3f} rel={err.max()/scale:.3e}")

    # full-output check
    got_a1, got_a2 = got[:, :H], got[:, H:]
    G_got = np.concatenate([own, got_a1, got_a2, own * got_a1, own * got_a2], -1)
    G_want = np.concatenate([own, A1, A2, own * A1, own * A2], -1)
    gerr = np.abs(G_got - G_want)
    gscale = np.abs(G_want).max()
    print(f"G: absmax={gerr.max():.3e} scale={gscale:.2f} rel={gerr.max()/gscale:.3e}")

